# revision 1
# baseline (speedup 1.0000x reference)
"""3-layer GCN (GCNConv x3) on 8 TRN2 NeuronCores via Bass/Tile.

Math: per layer  out = A_hat @ (x @ W) + b  with A_hat = D^-1/2 (A+I) D^-1/2.
By linearity we compute Z = A_hat_w @ x first (weighted scatter-add done as
dense matmuls against host-built selection tiles), then h = Z @ W + b (+relu).

Sharding: 1D node partition. Nodes padded to 50176 = 8 cores x 49 blocks x 128.
Each core aggregates + transforms its 49 destination blocks; source features
for layers 2/3 are replicated via AllGather between layers. Layer 1 gathers
from the (replicated) input x directly.

Per-core, per dst-block of 128 nodes: edges (sorted by dst) are chunked into
groups of 128. For each chunk, an indirect DMA gathers the 128 source rows
into an SBUF tile M [128e, 128f]; the PE accumulates
Z^T[f, d] += sum_e M[e, f] * S^T[e, d] over chunks, where S^T carries the
edge normalization weights (one nonzero per row). The GEMM consumes Z^T
feature-major with W stationary; ACT fuses bias+relu reading PSUM.
"""

import numpy as np

N = 50000
D = 128
P = 128
NCORES = 8
BLK = 49                  # dst blocks per core
PER = BLK * P             # 6272 nodes per core
NPAD = NCORES * PER       # 50176

_CACHE = {}


def _prep_graph(edge_index):
    """Host index preprocessing: sort edges by dst, build per-core gather
    indices and selection tiles. Returns (idxT[NC,P,NSUB], S[NC,NSUB*P,P],
    K_sub)."""
    src = np.concatenate([edge_index[0].astype(np.int64),
                          np.arange(N, dtype=np.int64)])
    dst = np.concatenate([edge_index[1].astype(np.int64),
                          np.arange(N, dtype=np.int64)])
    deg = np.bincount(dst, minlength=N).astype(np.float64)
    dinv = (1.0 / np.sqrt(deg)).astype(np.float32)
    w = (dinv[src] * dinv[dst]).astype(np.float32)

    order = np.argsort(dst, kind="stable")
    src, dst, w = src[order], dst[order], w[order]

    gblk = dst // P                                  # global block 0..390
    counts = np.bincount(gblk, minlength=NCORES * BLK)
    block_starts = np.concatenate([[0], np.cumsum(counts)])
    K_sub = int(np.ceil(counts.max() / P))
    NSUB = BLK * K_sub

    j = np.arange(len(dst)) - block_starts[gblk]     # rank within block
    core = gblk // BLK
    b_loc = gblk % BLK
    sub = b_loc * K_sub + j // P                     # subchunk within core
    lane = j % P
    d_loc = dst % P

    idxT = np.zeros((NCORES, P, NSUB), np.int32)
    idxT[core, lane, sub] = src
    S = np.zeros((NCORES, NSUB * P, P), np.float32)
    S[core, sub * P + lane, d_loc] = w
    return idxT, S, K_sub


def _build(K_sub):
    import concourse.bass as bass
    import concourse.mybir as mybir
    import concourse.tile as tile
    from concourse import bacc
    from concourse.masks import make_identity

    NSUB = BLK * K_sub
    f32 = mybir.dt.float32

    nc = bacc.Bacc("TRN2", target_bir_lowering=False, debug=False,
                   num_devices=NCORES)

    x_pad = nc.dram_tensor("x_pad", [NPAD, D], f32, kind="ExternalInput").ap()
    idx_in = nc.dram_tensor("idx", [P, NSUB], mybir.dt.int32,
                            kind="ExternalInput").ap()
    s_in = nc.dram_tensor("stiles", [NSUB * P, P], f32,
                          kind="ExternalInput").ap()
    Ws = [nc.dram_tensor(f"W{l}", [D, D], f32, kind="ExternalInput").ap()
          for l in (1, 2, 3)]
    bs = [nc.dram_tensor(f"b{l}", [D, 1], f32, kind="ExternalInput").ap()
          for l in (1, 2, 3)]
    out = nc.dram_tensor("out", [D, PER], f32, kind="ExternalOutput").ap()

    with tile.TileContext(nc) as tc:
        with tc.tile_pool(name="const", bufs=1) as cpool, \
             tc.tile_pool(name="idxp", bufs=1) as ipool, \
             tc.tile_pool(name="msg", bufs=8) as mpool, \
             tc.tile_pool(name="sel", bufs=8) as spool, \
             tc.tile_pool(name="work", bufs=3) as wpool, \
             tc.tile_pool(name="pz", bufs=2, space="PSUM") as pz, \
             tc.tile_pool(name="ph", bufs=2, space="PSUM") as ph, \
             tc.tile_pool(name="pt", bufs=2, space="PSUM") as pt, \
             tc.tile_pool(name="dram", bufs=1, space="DRAM") as dram:

            ident = cpool.tile([P, P], f32)
            make_identity(nc, ident[:])
            w_t = []
            b_t = []
            for l in range(3):
                wt = cpool.tile([D, D], f32, name=f"wt{l}")
                nc.sync.dma_start(out=wt[:], in_=Ws[l][:])
                bt = cpool.tile([D, 1], f32, name=f"bt{l}")
                nc.sync.dma_start(out=bt[:], in_=bs[l][:])
                w_t.append(wt)
                b_t.append(bt)
            idx_sb = ipool.tile([P, NSUB], mybir.dt.int32)
            nc.sync.dma_start(out=idx_sb[:], in_=idx_in[:])

            h_full = [None, None]
            ag_in = [None, None]
            for l in range(2):
                ag_in[l] = dram.tile([PER, D], f32, name=f"ag_in{l}")
                h_full[l] = dram.tile([NPAD, D], f32, addr_space="Shared",
                                      name=f"h_full{l}")

            for l in range(3):
                table = x_pad if l == 0 else h_full[l - 1][:]
                for b in range(BLK):
                    zt_ps = pz.tile([P, P], f32, space="PSUM", tag="zt")
                    for k in range(K_sub):
                        s = b * K_sub + k
                        m_t = mpool.tile([P, P], f32, tag="m")
                        nc.gpsimd.indirect_dma_start(
                            out=m_t[:], out_offset=None, in_=table,
                            in_offset=bass.IndirectOffsetOnAxis(
                                ap=idx_sb[:, s:s + 1], axis=0),
                        )
                        s_t = spool.tile([P, P], f32, tag="s")
                        nc.sync.dma_start(out=s_t[:],
                                          in_=s_in[s * P:(s + 1) * P, :])
                        nc.tensor.matmul(out=zt_ps[:], lhsT=m_t[:], rhs=s_t[:],
                                         start=(k == 0), stop=(k == K_sub - 1))
                    z_sb = wpool.tile([P, P], f32, tag="z")
                    nc.vector.tensor_copy(out=z_sb[:], in_=zt_ps[:])
                    # h^T = W^T @ Z^T (+bias, relu on layers 0,1)
                    h_ps = ph.tile([P, P], f32, space="PSUM", tag="h")
                    nc.tensor.matmul(out=h_ps[:], lhsT=w_t[l][:], rhs=z_sb[:],
                                     start=True, stop=True)
                    h_sb = wpool.tile([P, P], f32, tag="hs")
                    func = (mybir.ActivationFunctionType.Relu if l < 2
                            else mybir.ActivationFunctionType.Identity)
                    nc.scalar.activation(h_sb[:], h_ps[:], func,
                                         bias=b_t[l][:])
                    if l < 2:
                        # node-major for the gather table of the next layer
                        t_ps = pt.tile([P, P], f32, space="PSUM", tag="t")
                        nc.tensor.transpose(out=t_ps[:], in_=h_sb[:],
                                            identity=ident[:])
                        ht_sb = wpool.tile([P, P], f32, tag="ht")
                        nc.vector.tensor_copy(out=ht_sb[:], in_=t_ps[:])
                        nc.sync.dma_start(
                            out=ag_in[l][b * P:(b + 1) * P, :], in_=ht_sb[:])
                    else:
                        nc.sync.dma_start(out=out[:, b * P:(b + 1) * P],
                                          in_=h_sb[:])
                if l < 2:
                    nc.gpsimd.collective_compute(
                        "AllGather", mybir.AluOpType.bypass,
                        replica_groups=[list(range(NCORES))],
                        ins=[ag_in[l].opt()], outs=[h_full[l].opt()],
                    )

    nc.compile()
    return nc


def _get_compiled(K_sub):
    if K_sub not in _CACHE:
        _CACHE[K_sub] = _build(K_sub)
    return _CACHE[K_sub]


def _make_in_maps(x, edge_index, W1, b1, W2, b2, W3, b3):
    idxT, S, K_sub = _prep_graph(np.asarray(edge_index))
    x_pad = np.zeros((NPAD, D), np.float32)
    x_pad[:N] = np.asarray(x, np.float32)
    common = {
        "x_pad": x_pad,
        "W1": np.asarray(W1, np.float32), "b1": np.asarray(b1, np.float32).reshape(D, 1),
        "W2": np.asarray(W2, np.float32), "b2": np.asarray(b2, np.float32).reshape(D, 1),
        "W3": np.asarray(W3, np.float32), "b3": np.asarray(b3, np.float32).reshape(D, 1),
    }
    in_maps = []
    for c in range(NCORES):
        m = dict(common)
        m["idx"] = idxT[c]
        m["stiles"] = S[c]
        in_maps.append(m)
    return in_maps, K_sub


def _install_profile_shim():
    """This image's antenv lacks axon_hooks; recreate the NTFF hook from
    the boot helper so trace=True works. Test-side only."""
    import sys
    import types
    try:
        import antenv.axon_hooks  # noqa: F401
        return
    except ImportError:
        pass
    sys.path.insert(0, "/root/.axon_site/trn_agent_boot")
    import trn_boot
    hook = trn_boot._ntff_profile_via_ctypes("/opt/axon/libaxon_pjrt.so")
    import antenv
    mod = types.ModuleType("antenv.axon_hooks")
    state = {"hook": hook}
    mod.get_axon_ntff_profile_hook = lambda: state["hook"]
    mod.set_axon_ntff_profile_hook = lambda h: state.update(hook=h)
    sys.modules["antenv.axon_hooks"] = mod
    antenv.axon_hooks = mod
    # no fish credentials in this container; keep artifacts local
    import concourse.bass_utils as bu
    bu.upload_artifacts = lambda tmpdir: "local://" + str(tmpdir)


def _run(in_maps, K_sub, trace=False, tmpdir=None):
    from concourse.bass_utils import run_bass_kernel_spmd
    if trace:
        _install_profile_shim()
    nc = _get_compiled(K_sub)
    res = run_bass_kernel_spmd(nc, in_maps, core_ids=list(range(NCORES)),
                               trace=trace, tmpdir=tmpdir)
    return res


def kernel(x, edge_index, W1, b1, W2, b2, W3, b3):
    in_maps, K_sub = _make_in_maps(x, edge_index, W1, b1, W2, b2, W3, b3)
    res = _run(in_maps, K_sub)
    parts = [res.results[c]["out"].T for c in range(NCORES)]
    return np.concatenate(parts, axis=0)[:N].astype(np.float32)


def kernel_profiled(x, edge_index, W1, b1, W2, b2, W3, b3, tmpdir=None):
    """Like kernel() but runs with NTFF tracing; returns (output, results)."""
    in_maps, K_sub = _make_in_maps(x, edge_index, W1, b1, W2, b2, W3, b3)
    res = _run(in_maps, K_sub, trace=True, tmpdir=tmpdir)
    parts = [res.results[c]["out"].T for c in range(NCORES)]
    return np.concatenate(parts, axis=0)[:N].astype(np.float32), res



# revision 3
# speedup vs baseline: 1.0273x; 1.0273x over previous
"""3-layer GCN (GCNConv x3) on 8 TRN2 NeuronCores via Bass/Tile.

Math: per layer  out = A_hat @ (x @ W) + b  with A_hat = D^-1/2 (A+I) D^-1/2.
By linearity we compute Z = A_hat_w @ x first (weighted scatter-add done as
matmuls against one-hot selection tiles), then h = Z @ W + b (+relu).

The edge weight w = dinv[src]*dinv[dst] is factored out of the selection
tiles: dinv[src] is folded into the gather table rows (tables store
h~ = relu(h)*dinv, and x~ = x*dinv is host-prepared), dinv[dst] is applied
to the aggregated Z block during the PSUM->SBUF drain (per-free-element
multiply with a broadcast dinv row tile). The selection tiles are then pure
0/1 one-hots, exactly representable in fp8e4 -> the whole per-core selection
structure (NSUB chunks of [128,128]) is cached in SBUF as fp8, loaded once.

Sharding: 1D node partition. Nodes padded to 50176 = 8 cores x 49 blocks x 128.
Each core aggregates + transforms its 49 destination blocks; source features
for layers 2/3 are replicated via AllGather (bf16) between layers. Layer 1
gathers from the (replicated, pre-scaled) input x directly.

Per-core, per dst-block of 128 nodes: edges (sorted by dst) are chunked into
K_sub groups of 128. ONE batched indirect DMA per block gathers all
K_sub*128 source rows (bf16) into an SBUF tile M [128e, K_sub*128] --
amortizing the ~1us SWDGE fixed cost that dominated the per-chunk version.
The PE accumulates Z^T[f, d] += sum_e M_k[e, f] * S_k[e, d] over chunks
(lhsT = bf16 messages, rhs = fp8 one-hot, 1 cycle/row). The GEMM consumes
Z^T feature-major with W (bf16) stationary; ACT fuses bias+relu from PSUM.
"""

import numpy as np

N = 50000
D = 128
P = 128
NCORES = 8
BLK = 49                  # dst blocks per core
PER = BLK * P             # 6272 nodes per core
NPAD = NCORES * PER       # 50176

_CACHE = {}


def _np_dt(name):
    import concourse.mybir as mybir
    return np.dtype(mybir.dt.np(getattr(mybir.dt, name)))


def _prep_graph(edge_index):
    """Host index preprocessing: sort edges by dst, build per-core gather
    indices, fp8 one-hot selection tiles and dinv scale tensors.
    Returns (idxT[NC,P,NSUB], S[NC,P,NSUB*P] fp8, dinv_rows[NC,P,BLK*P] bf16,
    dinv_cols[NC,P,BLK] f32, dinv[NPAD] f32, K_sub)."""
    bf16 = _np_dt("bfloat16")
    f8 = _np_dt("float8e4")

    src = np.concatenate([edge_index[0].astype(np.int64),
                          np.arange(N, dtype=np.int64)])
    dst = np.concatenate([edge_index[1].astype(np.int64),
                          np.arange(N, dtype=np.int64)])
    deg = np.bincount(dst, minlength=N).astype(np.float64)
    dinv = np.zeros(NPAD, np.float32)
    dinv[:N] = (1.0 / np.sqrt(deg)).astype(np.float32)

    order = np.argsort(dst, kind="stable")
    src, dst = src[order], dst[order]

    gblk = dst // P                                  # global block 0..391
    counts = np.bincount(gblk, minlength=NCORES * BLK)
    block_starts = np.concatenate([[0], np.cumsum(counts)])
    K_sub = int(np.ceil(counts.max() / P))
    NSUB = BLK * K_sub

    j = np.arange(len(dst)) - block_starts[gblk]     # rank within block
    core = gblk // BLK
    b_loc = gblk % BLK
    sub = b_loc * K_sub + j // P                     # subchunk within core
    lane = j % P
    d_loc = dst % P

    idxT = np.zeros((NCORES, P, NSUB), np.int32)
    idxT[core, lane, sub] = src
    S = np.zeros((NCORES, P, NSUB * P), f8)
    S[core, lane, sub * P + d_loc] = 1.0

    # per-block broadcast rows of dinv (dst scaling, applied along free dim)
    dv_blocks = dinv.reshape(NCORES, BLK * P)
    dinv_rows = np.broadcast_to(dv_blocks[:, None, :],
                                (NCORES, P, BLK * P)).astype(bf16)
    # per-block columns of dinv (table pre-scaling, applied per partition)
    dinv_cols = np.ascontiguousarray(
        dv_blocks.reshape(NCORES, BLK, P).transpose(0, 2, 1)).astype(np.float32)
    return idxT, S, dinv_rows, dinv_cols, dinv, K_sub


def _build(K_sub):
    import concourse.bass as bass
    import concourse.mybir as mybir
    import concourse.tile as tile
    from concourse import bacc
    from concourse.masks import make_identity

    NSUB = BLK * K_sub
    f32 = mybir.dt.float32
    bf16 = mybir.dt.bfloat16
    f8 = mybir.dt.float8e4

    nc = bacc.Bacc("TRN2", target_bir_lowering=False, debug=False,
                   num_devices=NCORES)

    x_pad = nc.dram_tensor("x_pad", [NPAD, D], bf16, kind="ExternalInput").ap()
    idx_in = nc.dram_tensor("idx", [P, NSUB], mybir.dt.int32,
                            kind="ExternalInput").ap()
    s_in = nc.dram_tensor("stiles", [P, NSUB * P], f8,
                          kind="ExternalInput").ap()
    dvr_in = nc.dram_tensor("dinv_rows", [P, BLK * P], bf16,
                            kind="ExternalInput").ap()
    dvc_in = nc.dram_tensor("dinv_cols", [P, BLK], f32,
                            kind="ExternalInput").ap()
    Ws = [nc.dram_tensor(f"W{l}", [D, D], bf16, kind="ExternalInput").ap()
          for l in (1, 2, 3)]
    bs = [nc.dram_tensor(f"b{l}", [D, 1], f32, kind="ExternalInput").ap()
          for l in (1, 2, 3)]
    out = nc.dram_tensor("out", [D, PER], f32, kind="ExternalOutput").ap()

    with tile.TileContext(nc) as tc:
        with tc.tile_pool(name="const", bufs=1) as cpool, \
             tc.tile_pool(name="msg", bufs=12) as mpool, \
             tc.tile_pool(name="zp", bufs=4) as zpool, \
             tc.tile_pool(name="hp", bufs=4) as hpool, \
             tc.tile_pool(name="htp", bufs=4) as htpool, \
             tc.tile_pool(name="hfp", bufs=2) as hfpool, \
             tc.tile_pool(name="pz", bufs=4, space="PSUM") as pz, \
             tc.tile_pool(name="ph", bufs=2, space="PSUM") as ph, \
             tc.tile_pool(name="pt", bufs=2, space="PSUM") as pt, \
             tc.tile_pool(name="dram", bufs=1, space="DRAM") as dram:

            ident = cpool.tile([P, P], bf16)
            make_identity(nc, ident[:])
            w_t = []
            b_t = []
            for l in range(3):
                wt = cpool.tile([D, D], bf16, name=f"wt{l}")
                nc.sync.dma_start(out=wt[:], in_=Ws[l][:])
                bt = cpool.tile([D, 1], f32, name=f"bt{l}")
                nc.sync.dma_start(out=bt[:], in_=bs[l][:])
                w_t.append(wt)
                b_t.append(bt)
            idx_sb = cpool.tile([P, NSUB], mybir.dt.int32, name="idx")
            nc.sync.dma_start(out=idx_sb[:], in_=idx_in[:])
            dvr_sb = cpool.tile([P, BLK * P], bf16, name="dvr")
            nc.sync.dma_start(out=dvr_sb[:], in_=dvr_in[:])
            dvc_sb = cpool.tile([P, BLK], f32, name="dvc")
            nc.sync.dma_start(out=dvc_sb[:], in_=dvc_in[:])
            # per-block fp8 one-hot cache, loaded once, reused by all layers
            s_t = []
            for b in range(BLK):
                st = cpool.tile([P, K_sub * P], f8, name=f"s{b}")
                nc.sync.dma_start(
                    out=st[:], in_=s_in[:, b * K_sub * P:(b + 1) * K_sub * P])
                s_t.append(st)

            h_full = [None, None]
            ag_in = [None, None]
            for l in range(2):
                ag_in[l] = dram.tile([PER, D], bf16, name=f"ag_in{l}")
                h_full[l] = dram.tile([NPAD, D], bf16, addr_space="Shared",
                                      name=f"h_full{l}")

            for l in range(3):
                table = x_pad if l == 0 else h_full[l - 1][:]
                # 1-block software pipeline: the GEMM/ACT/transpose tail of
                # block b-1 is issued after block b's aggregation matmuls so
                # the PE never stalls waiting on the DVE drain of z.
                pending = None

                def tail(args, l=l):
                    zt_ps, b = args
                    z_sb = zpool.tile([P, P], bf16, tag="z")
                    nc.vector.tensor_tensor(
                        out=z_sb[:], in0=zt_ps[:],
                        in1=dvr_sb[:, b * P:(b + 1) * P],
                        op=mybir.AluOpType.mult)
                    # h^T = W^T @ Z^T (+bias, relu on layers 0,1)
                    h_ps = ph.tile([P, P], f32, space="PSUM", tag="h")
                    nc.tensor.matmul(out=h_ps[:], lhsT=w_t[l][:], rhs=z_sb[:],
                                     start=True, stop=True)
                    if l < 2:
                        h_sb = hpool.tile([P, P], bf16, tag="hs")
                        nc.scalar.activation(h_sb[:], h_ps[:],
                                             mybir.ActivationFunctionType.Relu,
                                             bias=b_t[l][:])
                        # node-major + dinv pre-scale for next layer's table
                        t_ps = pt.tile([P, P], bf16, space="PSUM", tag="t")
                        nc.tensor.transpose(out=t_ps[:], in_=h_sb[:],
                                            identity=ident[:])
                        ht_sb = htpool.tile([P, P], bf16, tag="ht")
                        nc.scalar.mul(ht_sb[:], t_ps[:], dvc_sb[:, b:b + 1])
                        nc.sync.dma_start(
                            out=ag_in[l][b * P:(b + 1) * P, :], in_=ht_sb[:])
                    else:
                        h_sb = hfpool.tile([P, P], f32, tag="hf")
                        nc.scalar.activation(
                            h_sb[:], h_ps[:],
                            mybir.ActivationFunctionType.Identity,
                            bias=b_t[l][:])
                        nc.sync.dma_start(out=out[:, b * P:(b + 1) * P],
                                          in_=h_sb[:])

                for b in range(BLK):
                    m_t = mpool.tile([P, K_sub * P], bf16, tag="m")
                    # per-chunk [128,1]-offset gathers: multi-column offset
                    # APs mis-read offsets on HW for partitions >= 64
                    for k in range(K_sub):
                        nc.gpsimd.indirect_dma_start(
                            out=m_t[:, k * P:(k + 1) * P], out_offset=None,
                            in_=table,
                            in_offset=bass.IndirectOffsetOnAxis(
                                ap=idx_sb[:, b * K_sub + k:b * K_sub + k + 1],
                                axis=0),
                        )
                    zt_ps = pz.tile([P, P], f32, space="PSUM", tag="zt")
                    for k in range(K_sub):
                        nc.tensor.matmul(
                            out=zt_ps[:],
                            lhsT=m_t[:, k * P:(k + 1) * P],
                            rhs=s_t[b][:, k * P:(k + 1) * P],
                            start=(k == 0), stop=(k == K_sub - 1))
                    if pending is not None:
                        tail(pending)
                    pending = (zt_ps, b)
                tail(pending)

                if l < 2:
                    nc.gpsimd.collective_compute(
                        "AllGather", mybir.AluOpType.bypass,
                        replica_groups=[list(range(NCORES))],
                        ins=[ag_in[l].opt()], outs=[h_full[l].opt()],
                    )

    nc.compile()
    return nc


def _get_compiled(K_sub):
    if K_sub not in _CACHE:
        _CACHE[K_sub] = _build(K_sub)
    return _CACHE[K_sub]


def _make_in_maps(x, edge_index, W1, b1, W2, b2, W3, b3):
    bf16 = _np_dt("bfloat16")
    idxT, S, dinv_rows, dinv_cols, dinv, K_sub = _prep_graph(
        np.asarray(edge_index))
    x_pad = np.zeros((NPAD, D), np.float32)
    x_pad[:N] = np.asarray(x, np.float32)
    x_pad *= dinv[:, None]                    # fold dinv[src] into the table
    common = {
        "x_pad": x_pad.astype(bf16),
        "W1": np.asarray(W1, np.float32).astype(bf16),
        "W2": np.asarray(W2, np.float32).astype(bf16),
        "W3": np.asarray(W3, np.float32).astype(bf16),
        "b1": np.asarray(b1, np.float32).reshape(D, 1),
        "b2": np.asarray(b2, np.float32).reshape(D, 1),
        "b3": np.asarray(b3, np.float32).reshape(D, 1),
    }
    in_maps = []
    for c in range(NCORES):
        m = dict(common)
        m["idx"] = idxT[c]
        m["stiles"] = S[c]
        m["dinv_rows"] = dinv_rows[c]
        m["dinv_cols"] = dinv_cols[c]
        in_maps.append(m)
    return in_maps, K_sub


def _install_profile_shim():
    """This image's antenv lacks axon_hooks; recreate the NTFF hook from
    the boot helper so trace=True works. Test-side only."""
    import sys
    import types
    try:
        import antenv.axon_hooks  # noqa: F401
        return
    except ImportError:
        pass
    sys.path.insert(0, "/root/.axon_site/trn_agent_boot")
    import trn_boot
    hook = trn_boot._ntff_profile_via_ctypes("/opt/axon/libaxon_pjrt.so")
    import antenv
    mod = types.ModuleType("antenv.axon_hooks")
    state = {"hook": hook}
    mod.get_axon_ntff_profile_hook = lambda: state["hook"]
    mod.set_axon_ntff_profile_hook = lambda h: state.update(hook=h)
    sys.modules["antenv.axon_hooks"] = mod
    antenv.axon_hooks = mod
    # no fish credentials in this container; keep artifacts local
    import concourse.bass_utils as bu
    bu.upload_artifacts = lambda tmpdir: "local://" + str(tmpdir)


def _run(in_maps, K_sub, trace=False, tmpdir=None):
    from concourse.bass_utils import run_bass_kernel_spmd
    if trace:
        _install_profile_shim()
    nc = _get_compiled(K_sub)
    res = run_bass_kernel_spmd(nc, in_maps, core_ids=list(range(NCORES)),
                               trace=trace, tmpdir=tmpdir)
    return res


def kernel(x, edge_index, W1, b1, W2, b2, W3, b3):
    in_maps, K_sub = _make_in_maps(x, edge_index, W1, b1, W2, b2, W3, b3)
    res = _run(in_maps, K_sub)
    parts = [res.results[c]["out"].T for c in range(NCORES)]
    return np.concatenate(parts, axis=0)[:N].astype(np.float32)


def kernel_profiled(x, edge_index, W1, b1, W2, b2, W3, b3, tmpdir=None):
    """Like kernel() but runs with NTFF tracing; returns (output, results)."""
    in_maps, K_sub = _make_in_maps(x, edge_index, W1, b1, W2, b2, W3, b3)
    res = _run(in_maps, K_sub, trace=True, tmpdir=tmpdir)
    parts = [res.results[c]["out"].T for c in range(NCORES)]
    return np.concatenate(parts, axis=0)[:N].astype(np.float32), res
